# revision 23
# baseline (speedup 1.0000x reference)
"""DT-GRU layer Bass kernel for 8 TRN2 NeuronCores.

Problem: nn_DTGRULayer (B=256, T=256, ENC_IN=64, D_MODEL=512).
Strategy: data-parallel over batch (32 rows/core), weights replicated.

Per-core layout: activations are kept feature-major in SBUF as
[128 partitions, 4 k-tiles, 32 batch].  Every matmul is
out[j, b] += W[k, j].T @ act[k, b] with the weight tile [128K x 128M]
stationary (bf16) and the activation [128K x 32] moving (bf16), fp32
PSUM accumulation.  Elementwise/gate math is fp32 on DVE/ACT.

All sigmoids are computed as sigmoid(x) = (1 + tanh(x/2)) / 2 so the
scalar engine only ever needs the {tanh, exp} activation table
(`exp_and_others`) -- a table switch costs 1283 ns, and the naive
sigmoid/tanh/exp mix would pay two per step.  The /2 for the reset
gates is folded into pre-scaled copies of Una/Uhm on the host; the
(1+y)/2 for the update gates is folded into the DVE gate-combine ops.
Softmax over the feature dim uses a ones-vector matmul for the
cross-partition sum and a K=1 ones matmul to broadcast 1/sum; bma is
added via a K=2 matmul of hi/lo bf16 parts (exact in fp32).
"""

import os
import sys
import numpy as np

try:
    import concourse  # noqa: F401  (normally provided via the axon PYTHONPATH)
except ImportError:
    sys.path.insert(0, "/opt/trn_rl_repo")

import ml_dtypes

BF16 = ml_dtypes.bfloat16

ENC_IN = 64
D = 512
B_FULL = 256
T_FULL = 256
N_CORES = 8
BP = B_FULL // N_CORES          # 32 batch rows per core
KT = D // 128                   # 4 k-tiles of the feature dim
UNROLL = 12                     # loop-body unroll (must be multiple of 3)

# weight tile bookkeeping: name -> (K, n_cols)
WSPECS = [
    ("Wa", D, 2 * D), ("Ua", D, 2 * D),
    ("Wna", D, D), ("Una", D, D),
    ("WAttn", D, D), ("UAttn", D, D), ("VAttn", D, D),
    ("Wma", D, D), ("Uma", D, D),
    ("Um", D, 2 * D), ("Uhm", D, D),
    ("Wm", ENC_IN, 2 * D), ("Whm", ENC_IN, D),
]
# host-side prescales folded into the weights (reset-gate sigmoid /2)
WSCALE = {"Una": 0.5, "Uhm": 0.5}


def _wb_layout():
    """Column offsets of each weight tile inside the packed WB blob."""
    off = {}
    col = 0
    for name, k, n in WSPECS:
        nk = (k + 127) // 128
        nj = n // 128
        off[name] = (col, nk, nj, k)
        col += nk * nj * 128
    return off, col


WB_OFF, WB_COLS = _wb_layout()


def pack_weights(w):
    """Pack all weights (dict of fp32 arrays) into [128, WB_COLS] bf16."""
    blob = np.zeros((128, WB_COLS), dtype=BF16)
    for name, k, n in WSPECS:
        base, nk, nj, kk = WB_OFF[name]
        arr = np.asarray(w[name], dtype=np.float32) * WSCALE.get(name, 1.0)
        assert arr.shape == (kk, n)
        if kk == 128 * nk:
            t = arr.reshape(nk, 128, nj, 128).transpose(1, 0, 2, 3).reshape(128, nk * nj * 128)
            blob[:, base:base + nk * nj * 128] = t.astype(BF16)
        else:  # K=64
            t = arr.reshape(kk, nj * 128)
            blob[:kk, base:base + nj * 128] = t.astype(BF16)
    return blob


def build_nc(T=T_FULL, repeat=1):
    """Build the Bass/Tile program for one core (SPMD across 8).

    repeat > 1 re-runs the main recurrence loop `repeat` times (state is NOT
    reset between passes) -- used only for differential wall-clock timing.
    """
    import concourse.bass as bass
    import concourse.bacc as bacc
    import concourse.tile as tile
    from concourse import mybir

    dt = mybir.dt
    AF = mybir.ActivationFunctionType
    OP = mybir.AluOpType

    nc = bacc.Bacc("TRN2", target_bir_lowering=False, debug=False)

    # ---- DRAM I/O ----
    wb_d = nc.dram_tensor("wb", [128, WB_COLS], dt.bfloat16, kind="ExternalInput")
    xT_d = nc.dram_tensor("xT", [ENC_IN, T * BP], dt.bfloat16, kind="ExternalInput")
    bma2_d = nc.dram_tensor("bma2", [2, D], dt.bfloat16, kind="ExternalInput")
    ones_c_d = nc.dram_tensor("ones_c", [128, 1], dt.bfloat16, kind="ExternalInput")
    ones_r_d = nc.dram_tensor("ones_r", [1, 128], dt.bfloat16, kind="ExternalInput")
    ones2_d = nc.dram_tensor("ones2", [2, BP], dt.bfloat16, kind="ExternalInput")

    hs_d = nc.dram_tensor("hs", [T, 128, KT * BP], dt.float32, kind="ExternalOutput")
    hT_d = nc.dram_tensor("hT", [128, KT * BP], dt.float32, kind="ExternalOutput")
    NT_d = nc.dram_tensor("NT", [128, KT * BP], dt.float32, kind="ExternalOutput")

    f32 = dt.float32
    bf16 = dt.bfloat16

    with tile.TileContext(nc) as tc:
        with (
            tc.tile_pool(name="persist", bufs=1) as pp,
            tc.tile_pool(name="work", bufs=2) as wp,
            tc.tile_pool(name="psA", bufs=2, space="PSUM") as psA,
            tc.tile_pool(name="psB", bufs=2, space="PSUM") as psB,
            tc.tile_pool(name="psB2", bufs=1, space="PSUM") as psB2,
            tc.tile_pool(name="psC", bufs=1, space="PSUM") as psC,
        ):
            # ---- persistent SBUF ----
            wb = pp.tile([128, WB_COLS], bf16, tag="wb")
            xT = pp.tile([ENC_IN, T * BP], bf16, tag="xT")
            bma2 = pp.tile([2, D], bf16, tag="bma2")
            ones_c = pp.tile([128, 1], bf16, tag="ones_c")
            ones_r = pp.tile([1, 128], bf16, tag="ones_r")
            ones2 = pp.tile([2, BP], bf16, tag="ones2")

            nc.sync.dma_start(out=wb[:], in_=wb_d[:])
            nc.sync.dma_start(out=xT[:], in_=xT_d[:])
            nc.sync.dma_start(out=bma2[:], in_=bma2_d[:])
            nc.sync.dma_start(out=ones_c[:], in_=ones_c_d[:])
            nc.sync.dma_start(out=ones_r[:], in_=ones_r_d[:])
            nc.sync.dma_start(out=ones2[:], in_=ones2_d[:])

            h0 = pp.tile([128, KT, BP], f32, tag="h0")
            h1 = pp.tile([128, KT, BP], f32, tag="h1")
            h2 = pp.tile([128, KT, BP], f32, tag="h2")
            Nst = pp.tile([128, KT, BP], f32, tag="Nst")
            h_bf = pp.tile([128, KT, BP], bf16, tag="h_bf")
            N_bf = pp.tile([128, KT, BP], bf16, tag="N_bf")

            for tl in (h0, h1, h2, Nst):
                nc.vector.memset(tl[:], 0.0)
            nc.vector.memset(h_bf[:], 0.0)
            nc.vector.memset(N_bf[:], 0.0)
            zeroC = pp.tile([1, 2 * BP], bf16, tag="zeroC")
            nc.vector.memset(zeroC[:], 0.0)

            def wtile(name, kt, jt):
                base, nk, nj, kk = WB_OFF[name]
                c = base + (kt * nj + jt) * 128
                if kk >= 128:
                    return wb[:, c:c + 128]
                return wb[:kk, c:c + 128]

            def mm(out_ap, lhsT, rhs, start, stop):
                nc.tensor.matmul(out_ap, lhsT, rhs, start=start, stop=stop)

            class Grp:
                """One PSUM bank holding one or more accumulation regions.

                HW semantics: the first matmul into a bank must carry
                start=True (clears the whole bank's has_written bits); every
                later matmul into the bank must NOT, so disjoint regions
                lazily overwrite-then-accumulate.  stop=True goes on the
                final matmul emitted into the bank.
                """

                def __init__(self, tile_ap):
                    self.t = tile_ap
                    self.started = False

                def mm(self, out_ap, lhsT, rhs, last=False):
                    # skip_group_check: banks hold several logical regions;
                    # correctness comes from the single-start-per-bank rule
                    # plus per-element has_written (see class docstring).
                    nc.tensor.matmul(out_ap, lhsT, rhs,
                                     start=not self.started, stop=last,
                                     skip_group_check=True)
                    self.started = True

            def start_gA_half(j0, name):
                """One gates_A half (z: j0=0, r: j0=KT) + its Ua matmuls."""
                g = Grp(psA.tile([128, KT, BP], f32, tag="gA", name=name))
                for jt in range(KT):
                    for kt in range(KT):
                        g.mm(g.t[:, jt, :], wtile("Ua", kt, j0 + jt),
                             N_bf[:, kt, :])
                return g

            def start_gA():
                return (start_gA_half(0, "gAz"), start_gA_half(KT, "gAr"))

            def start_gM(xsl):
                """gates_M z/r psum tiles + their x@Wm matmuls (no deps)."""
                gz = Grp(psA.tile([128, KT, BP], f32, tag="gM", name="gMz"))
                gr = Grp(psA.tile([128, KT, BP], f32, tag="gM", name="gMr"))
                for g, j0 in ((gz, 0), (gr, KT)):
                    for jt in range(KT):
                        g.mm(g.t[:, jt, :], wtile("Wm", 0, j0 + jt), xsl)
                return gz, gr

            def xslice(idx):
                if isinstance(idx, int):
                    return xT[:, idx * BP:(idx + 1) * BP]
                return xT[:, bass.ds(idx * BP, BP)]

            def emit_step(idx, h_prev, h_pp, h_next, gA, gM, next_idx):
                """One recurrence step; idx is a python int or ScalarValue.

                gA/gM are (z, r) Grp pairs holding the Ua / x@Wm partial
                accumulations, hoisted into earlier stall windows.  The PE is
                in-order, so emission order IS the PE schedule: chain-critical
                matmul groups are emitted immediately after their producers,
                and independent "fill" groups (WAttn/bias/Uma on h, Wna on d,
                Ua/Wm of step t+1) are sized to the ACT/DVE stall windows
                between them.
                """
                gAz, gAr = gA
                gMz, gMr = gM

                # d_t = h_{t-1} - h_{t-2} (bf16 for PE)
                d_bf = wp.tile([128, KT, BP], bf16, tag="d_bf")
                nc.vector.tensor_sub(d_bf[:], h_prev[:], h_pp[:])

                # chain: gates_A r half (it gates hat_N via Nr)
                for jt in range(KT):
                    for kt in range(KT):
                        gAr.mm(gAr.t[:, jt, :], wtile("Wa", kt, KT + jt),
                               d_bf[:, kt, :],
                               last=(jt == KT - 1) and (kt == KT - 1))
                yA_r = wp.tile([128, KT, BP], f32, tag="yA_r")
                nc.scalar.activation(yA_r[:], gAr.t[:], AF.Tanh, scale=0.5)

                # B1 bank: [scp | hNp | scv] regions.  Wna (fill, d-ready)
                # runs under the yA_r window.
                B1 = Grp(psB.tile([128, 3, KT, BP], f32, tag="B1", name="B1"))
                scp = B1.t[:, 0]
                hNp = B1.t[:, 1]
                scv = B1.t[:, 2]
                for jt in range(KT):
                    for kt in range(KT):
                        B1.mm(hNp[:, jt, :], wtile("Wna", kt, jt),
                              d_bf[:, kt, :])

                Nr_bf = wp.tile([128, KT, BP], bf16, tag="Nr_bf")
                nc.vector.scalar_tensor_tensor(Nr_bf[:], yA_r[:], 1.0, Nst[:],
                                               OP.add, OP.mult)

                # gates_A z half under the Nr window
                for jt in range(KT):
                    for kt in range(KT):
                        gAz.mm(gAz.t[:, jt, :], wtile("Wa", kt, jt),
                               d_bf[:, kt, :],
                               last=(jt == KT - 1) and (kt == KT - 1))
                yA_z = wp.tile([128, KT, BP], f32, tag="yA_z")
                nc.scalar.activation(yA_z[:], gAz.t[:], AF.Tanh, scale=0.5)

                # chain: hat_N = tanh(d @ Wna + (N*sig(rA)) @ Una)
                for jt in range(KT):
                    for kt in range(KT):
                        B1.mm(hNp[:, jt, :], wtile("Una", kt, jt),
                              Nr_bf[:, kt, :])
                hatN = wp.tile([128, KT, BP], f32, tag="hatN")
                nc.scalar.activation(hatN[:], hNp[:], AF.Tanh)

                # fill under the hatN + N-update window: WAttn, bma, Uma
                for jt in range(KT):
                    for kt in range(KT):
                        B1.mm(scp[:, jt, :], wtile("WAttn", kt, jt),
                              h_bf[:, kt, :])
                B2 = Grp(psB2.tile([128, 2, KT, BP], f32, tag="B2", name="B2"))
                mp = B2.t[:, 0]
                hhp = B2.t[:, 1]
                for jt in range(KT):
                    B2.mm(mp[:, jt, :], bma2[:, jt * 128:(jt + 1) * 128],
                          ones2[:])
                for jt in range(KT):
                    for kt in range(KT):
                        B2.mm(mp[:, jt, :], wtile("Uma", kt, jt),
                              h_bf[:, kt, :])

                # N = N + sig(zA)*(hatN - N) = N + 0.5*(1+yA_z)*(hatN - N)
                tmp = wp.tile([128, KT, BP], f32, tag="tmp")
                nc.vector.tensor_sub(tmp[:], hatN[:], Nst[:])
                nc.vector.scalar_tensor_tensor(tmp[:], yA_z[:], 1.0, tmp[:],
                                               OP.add, OP.mult)
                nc.vector.scalar_tensor_tensor(Nst[:], tmp[:], 0.5, Nst[:],
                                               OP.mult, OP.add)
                nc.vector.tensor_copy(N_bf[:], Nst[:])

                # chain: score = tanh(h @ WAttn + N_new @ UAttn) @ VAttn
                for jt in range(KT):
                    for kt in range(KT):
                        B1.mm(scp[:, jt, :], wtile("UAttn", kt, jt),
                              N_bf[:, kt, :])
                ts_bf = wp.tile([128, KT, BP], bf16, tag="ts_bf")
                nc.scalar.activation(ts_bf[:], scp[:], AF.Tanh)

                # fill the ts window with half of next step's Ua
                gAr_next = start_gA_half(KT, "gAr") if next_idx is not None else None

                for jt in range(KT):
                    for kt in range(KT):
                        B1.mm(scv[:, jt, :], wtile("VAttn", kt, jt),
                              ts_bf[:, kt, :],
                              last=(jt == KT - 1) and (kt == KT - 1))

                # softmax over features (partition dim); scores are O(1) so no
                # max-subtraction is needed
                es_bf = wp.tile([128, KT, BP], bf16, tag="es_bf")
                nc.scalar.activation(es_bf[:], scv[:], AF.Exp)

                # fill the exp window with the other Ua half
                gAz_next = start_gA_half(0, "gAz") if next_idx is not None else None

                C = Grp(psC.tile([128, 2 * BP], f32, tag="C", name="Cs"))
                s_ps = C.t[0:1, 0:BP]
                r_ps = C.t[:, BP:2 * BP]
                # open the bank with a full-partition zeroing matmul so every
                # byte's has_written bit is set before the M=1 sum matmuls
                C.mm(C.t[:, :], ones_r[:], zeroC[:])
                for kt in range(KT):
                    C.mm(s_ps, ones_c[:], es_bf[:, kt, :])
                recip_bf = wp.tile([1, BP], bf16, tag="recip_bf")
                with nc.allow_low_precision("softmax denom recip feeds bf16 matmul"):
                    nc.vector.reciprocal(recip_bf[:], s_ps)
                C.mm(r_ps, ones_r[:], recip_bf[:], last=True)

                # Omega = softmax(score) * N_new; om_un runs under the
                # bcast-matmul window on the DVE
                om_un = wp.tile([128, KT, BP], f32, tag="om_un")
                nc.vector.tensor_mul(om_un[:], es_bf[:], Nst[:])
                om_bf = wp.tile([128, KT, BP], bf16, tag="om_bf")
                nc.vector.tensor_mul(
                    om_bf[:], om_un[:],
                    r_ps.unsqueeze(1).broadcast_to([128, KT, BP]))

                # chain: m = tanh(Omega @ Wma + h @ Uma + bma); bf16 (PE-only)
                for jt in range(KT):
                    for kt in range(KT):
                        B2.mm(mp[:, jt, :], wtile("Wma", kt, jt),
                              om_bf[:, kt, :])
                m_bf = wp.tile([128, KT, BP], bf16, tag="m_bf")
                nc.scalar.activation(m_bf[:], mp[:], AF.Tanh)

                # chain: gates_M r half (it gates hat_h via mr)
                for jt in range(KT):
                    for kt in range(KT):
                        gMr.mm(gMr.t[:, jt, :], wtile("Um", kt, KT + jt),
                               m_bf[:, kt, :],
                               last=(jt == KT - 1) and (kt == KT - 1))
                yM_r = wp.tile([128, KT, BP], f32, tag="yM_r")
                nc.scalar.activation(yM_r[:], gMr.t[:], AF.Tanh, scale=0.5)

                # fill the yM_r/mr window: x@Whm and the gates_M z half
                for jt in range(KT):
                    B2.mm(hhp[:, jt, :], wtile("Whm", 0, jt), xslice(idx))
                for jt in range(KT):
                    for kt in range(KT):
                        gMz.mm(gMz.t[:, jt, :], wtile("Um", kt, jt),
                               m_bf[:, kt, :],
                               last=(jt == KT - 1) and (kt == KT - 1))

                mr_bf = wp.tile([128, KT, BP], bf16, tag="mr_bf")
                nc.vector.scalar_tensor_tensor(mr_bf[:], yM_r[:], 1.0, m_bf[:],
                                               OP.add, OP.mult)
                yM_z = wp.tile([128, KT, BP], f32, tag="yM_z")
                nc.scalar.activation(yM_z[:], gMz.t[:], AF.Tanh, scale=0.5)

                # chain: hat_h = tanh(x @ Whm + (m*sig(rM)) @ Uhm)
                for jt in range(KT):
                    for kt in range(KT):
                        B2.mm(hhp[:, jt, :], wtile("Uhm", kt, jt),
                              mr_bf[:, kt, :],
                              last=(jt == KT - 1) and (kt == KT - 1))
                hath = wp.tile([128, KT, BP], f32, tag="hath")
                nc.scalar.activation(hath[:], hhp[:], AF.Tanh)

                # h_next = h + sig(zM)*(hath - h)
                tmp2 = wp.tile([128, KT, BP], f32, tag="tmp2")
                nc.vector.tensor_sub(tmp2[:], hath[:], h_prev[:])
                nc.vector.scalar_tensor_tensor(tmp2[:], yM_z[:], 1.0, tmp2[:],
                                               OP.add, OP.mult)
                nc.vector.scalar_tensor_tensor(h_next[:], tmp2[:], 0.5,
                                               h_prev[:], OP.mult, OP.add)
                nc.vector.tensor_copy(h_bf[:], h_next[:])

                # stream hidden state out
                if isinstance(idx, int):
                    nc.sync.dma_start(out=hs_d[idx], in_=h_next[:])
                else:
                    nc.sync.dma_start(out=hs_d[bass.ds(idx, 1)], in_=h_next[:])

                # tail fill: next step's x @ Wm
                gM_next = start_gM(xslice(next_idx)) if next_idx is not None else None
                gA_next = (gAz_next, gAr_next) if next_idx is not None else None
                return gA_next, gM_next

            rot = [h0, h1, h2]  # h_{t-1}, h_{t-2}, write-slot

            n_loop = (T // UNROLL) if T >= UNROLL else 0
            tail = T - n_loop * UNROLL

            def run_steps(idx0, n, dynamic_base=None):
                """Emit n consecutive steps starting at index expression."""
                nonlocal rot

                def mkidx(u):
                    return idx0 + u if dynamic_base is None else dynamic_base + u

                gA = start_gA()
                gM = start_gM(xslice(mkidx(0)))
                for u in range(n):
                    nxt = mkidx(u + 1) if u < n - 1 else None
                    gA, gM = emit_step(mkidx(u), rot[0], rot[1], rot[2],
                                       gA, gM, nxt)
                    rot = [rot[2], rot[0], rot[1]]

            def main_loop():
                if n_loop > 0:
                    with tc.For_i(0, n_loop, 1,
                                  hint_engines=(mybir.EngineType.PE,)) as iv:
                        run_steps(0, UNROLL, dynamic_base=iv * UNROLL)

            if repeat == 1:
                main_loop()
            else:
                with tc.For_i(0, repeat, 1) as _rep:
                    main_loop()
            if tail:
                run_steps(n_loop * UNROLL, tail)

            nc.sync.dma_start(out=hT_d[:], in_=rot[0][:])
            nc.sync.dma_start(out=NT_d[:], in_=Nst[:])

    nc.compile()
    return nc


def make_in_maps(inputs, T=T_FULL):
    """Shard/pack full inputs -> per-core in_maps."""
    wb = pack_weights(inputs)
    bma = np.asarray(inputs["bma"], np.float32).reshape(D)
    bma_hi = bma.astype(BF16)
    bma_lo = (bma - bma_hi.astype(np.float32)).astype(BF16)
    bma2 = np.stack([bma_hi, bma_lo], axis=0)          # [2, D]
    ones_c = np.ones((128, 1), dtype=BF16)
    ones_r = np.ones((1, 128), dtype=BF16)
    ones2 = np.ones((2, BP), dtype=BF16)
    x = np.asarray(inputs["x"], np.float32)
    in_maps = []
    for c in range(N_CORES):
        xc = x[c * BP:(c + 1) * BP, :T]                    # [BP, T, 64]
        xTc = np.ascontiguousarray(xc.transpose(2, 1, 0))  # [64, T, BP]
        in_maps.append({
            "wb": wb,
            "xT": xTc.astype(BF16).reshape(ENC_IN, T * BP),
            "bma2": bma2,
            "ones_c": ones_c,
            "ones_r": ones_r,
            "ones2": ones2,
        })
    return in_maps


def decode_outputs(results, T=T_FULL):
    """Per-core results -> (hidden_seq [B,T,D], (h_T [B,D], N_T [B,D]))."""
    hs_parts, hT_parts, NT_parts = [], [], []
    for r in results:
        hs = r["hs"].reshape(T, 128, KT, BP)
        hs_parts.append(np.ascontiguousarray(hs.transpose(3, 0, 2, 1)).reshape(BP, T, D))
        hT = r["hT"].reshape(128, KT, BP)
        hT_parts.append(np.ascontiguousarray(hT.transpose(2, 1, 0)).reshape(BP, D))
        NT = r["NT"].reshape(128, KT, BP)
        NT_parts.append(np.ascontiguousarray(NT.transpose(2, 1, 0)).reshape(BP, D))
    hidden = np.concatenate(hs_parts, axis=0)
    h_T = np.concatenate(hT_parts, axis=0)
    N_T = np.concatenate(NT_parts, axis=0)
    return hidden, (h_T, N_T)


_NC_CACHE = {}


def kernel(**inputs):
    from concourse.bass_utils import run_bass_kernel_spmd

    T = T_FULL
    if T not in _NC_CACHE:
        _NC_CACHE[T] = build_nc(T)
    nc = _NC_CACHE[T]
    in_maps = make_in_maps(inputs, T)
    res = run_bass_kernel_spmd(nc, in_maps, list(range(N_CORES)))
    return decode_outputs(res.results, T)
